# revision 14
# baseline (speedup 1.0000x reference)
"""Trainium2 Bass kernel for nn_EnsembleHead (FC -> LSTM -> linear -> softmax over time).

Contract: kernel(**inputs) takes FULL unsharded numpy inputs (keys as in
setup_inputs) and returns the FULL (1024, 512) float32 output.

Strategy (hardcoded, self-contained):
  - Sequence-parallel over 8 NeuronCores: the 512-step scan is split into 8
    slices of 64 owned steps; every core runs the FULL batch (1024 rows) for
    its slice, 68 steps total (4 warmup + 64 owned). LSTM state forgetting
    decays cold-start error; 4 warmup steps keep rel err ~7e-4 (Frobenius).
  - SPMD-uniform warmup: a "delta" row carries a -30 bias into every gate,
    pinning h=c=0; slice 0 sets delta=1 for its prefix steps.
  - Batch-half stacking: batch is split into 2 pairs x 2 halves of 256.
    Block-diagonal lhsT weights ([[W;0],[0;W]]) compute each gate for BOTH
    halves in one M=128 matmul, yielding per-gate PSUM tiles [q_h0; q_h1]
    stacked on partition halves. Every elementwise op then runs on all 128
    partitions (half the free-dim cost vs a 64-partition layout).
  - Gate PSUM per pair: G_p = [128, 1024] f32, cols [i|f|g|o]. One sigmoid
    ACT covers all four gates (g-rows pre-scaled by 2: tanh(z)=2*sig(2z)-1).
  - x-side gate matmuls (K=64 block-diag; x for the two halves stacked on
    partitions 0:32/32:64) accumulate start=True one step ahead of the
    h-side matmuls (start=False), keeping them off the recurrent chain.
  - Vector chain per pair: fixup ts (tg=2s-1), fused [si|sf]*[tg|c] tt
    (FD=512), c-add (FD=256), h-mult (FD=256) -- all [128, *].
  - Per-step logits (h @ W_last.T) are emitted one step deferred so the
    N=1 matmuls fill PE slack; they accumulate into one PSUM bank.
  - Tail: AllGather of logit blocks, then every core computes the softmax
    over time for all 1024 rows; the host reads core 0's copy.
"""
import numpy as np
import ml_dtypes

import concourse.bacc as bacc
import concourse.mybir as mybir
import concourse.tile as tile
from concourse.bass_utils import run_bass_kernel_spmd

F32 = mybir.dt.float32
BF16 = mybir.dt.bfloat16
AF = mybir.ActivationFunctionType
ALU = mybir.AluOpType

B, N, DIN, H = 1024, 512, 30, 64
NCORES = 8
SQ = 8                    # sequence slices
WARM = 4                  # warmup steps
OWN = N // SQ             # 64 owned steps per core
SPC = OWN + WARM          # 68 steps per core
PAIRS = 2
PW = 256                  # half-width (batch columns per partition half)
XR = 32                   # x rows per half: x(30), ones, delta
T = 16                    # timesteps per x-chunk
CLEN = [16, 16, 16, 16, 4]    # per-chunk step counts (sum = SPC)
CS = [0, 16, 32, 48, 64]      # chunk start steps
NCH = len(CLEN)
NG = B // 128             # batch groups of 128 rows for logits

_CACHE: dict = {}


def _build():
    nc = bacc.Bacc("TRN2", target_bir_lowering=False, debug=False, num_devices=NCORES)
    xt = nc.dram_tensor("xt", [2 * XR, SPC * 2 * PW], BF16, kind="ExternalInput")
    wh = nc.dram_tensor("wh", [128, 4 * 128], BF16, kind="ExternalInput")
    wx = nc.dram_tensor("wx", [2 * XR, 4 * 128], BF16, kind="ExternalInput")
    wl = nc.dram_tensor("wl", [128, 1], BF16, kind="ExternalInput")
    y = nc.dram_tensor("yh", [B, N], F32, kind="ExternalOutput")

    with tile.TileContext(nc) as tc:
        with (
            tc.tile_pool(name="const", bufs=1) as cpool,
            tc.tile_pool(name="bufp", bufs=1) as bufp,
            tc.tile_pool(name="state", bufs=1) as spool,
            tc.tile_pool(name="vh", bufs=3) as vpool,
            tc.tile_pool(name="work", bufs=2) as wpool,
            tc.tile_pool(name="gp", bufs=1, space="PSUM") as gpool,
            tc.tile_pool(name="lp", bufs=1, space="PSUM") as lpool,
            tc.tile_pool(name="dram", bufs=1, space="DRAM") as dpool,
        ):
            # tiny dummy activation: walrus emits the ACT table load before
            # the first ACTIVATE, so this pulls the ~2.6us sigmoid/tanh table
            # load into the head, overlapped with the input DMAs
            dum0 = cpool.tile([128, 1], F32, tag="dum0")
            dum1 = cpool.tile([128, 1], F32, tag="dum1")
            nc.gpsimd.memset(dum0[:], 0.0)
            nc.scalar.activation(dum1[:], dum0[:], AF.Sigmoid)

            wht = cpool.tile([128, 4 * 128], BF16, tag="wh")
            wxt = cpool.tile([2 * XR, 4 * 128], BF16, tag="wx")
            wlt = cpool.tile([128, 1], BF16, tag="wl")
            nc.sync.dma_start(wht[:], wh.ap())
            nc.sync.dma_start(wxt[:], wx.ap())
            nc.sync.dma_start(wlt[:], wl.ap())

            bufs = [bufp.tile([2 * XR, T * 2 * PW], BF16, tag=f"buf{i}",
                              name=f"buf{i}") for i in range(2)]
            # U_p: [tg (0:PW) | c (PW:2PW)] persistent per pair
            ucs = [spool.tile([128, 2 * PW], BF16, tag=f"uc{p}", name=f"uc{p}")
                   for p in range(PAIRS)]
            gps = [gpool.tile([128, 4 * PW], F32, tag=f"gp{p}", name=f"gpt{p}")
                   for p in range(PAIRS)]
            logits = lpool.tile([128, OWN * NG], F32, tag="logits")
            # gather pieces: (tloc start, tloc end, trigger step or None=end)
            PIECES = [(0, 32, 44), (32, 56, 60), (56, 64, None)]
            cins = [dpool.tile([128, (b - a) * NG], F32, tag=f"cin{i}", name=f"cin{i}")
                    for i, (a, b, _) in enumerate(PIECES)]
            couts = [dpool.tile([SQ * 128, (b - a) * NG], F32, tag=f"cout{i}",
                                name=f"cout{i}")
                     for i, (a, b, _) in enumerate(PIECES)]
            fls = [wpool.tile([128, N], F32, tag=f"fl{g}", name=f"fl{g}", bufs=1)
                   for g in range(NG)]

            def emit_gather(i):
                a, b, _ = PIECES[i]
                w = b - a
                lsb = wpool.tile([128, w * NG], F32, tag=f"lsb{i}", name=f"lsb{i}",
                                 bufs=1)
                nc.vector.tensor_copy(
                    lsb.rearrange("p (g t) -> p g t", g=NG),
                    logits.rearrange("p (g t) -> p g t", g=NG)[:, :, a:b],
                )
                nc.sync.dma_start(cins[i][:], lsb[:])
                nc.gpsimd.collective_compute(
                    "AllGather",
                    ALU.bypass,
                    replica_groups=[[q for q in range(SQ)]],
                    ins=[cins[i].opt()],
                    outs=[couts[i].opt()],
                )
                for g in range(NG):
                    fl3 = fls[g].rearrange("p (q t) -> p q t", q=SQ)
                    srci = couts[i].rearrange("(q p) n -> p q n", p=128)[
                        :, :, g * w : (g + 1) * w
                    ]
                    nc.sync.dma_start(fl3[:, :, a:b], srci)

            def emit_logits(sl, vt):
                # logits for owned step sl from history tile vt
                tloc = sl - WARM
                for g in range(NG):
                    p, j, m0 = g // 4, (g % 4) // 2, (g % 2) * 128
                    nc.tensor.matmul(
                        logits[:, g * OWN + tloc : g * OWN + tloc + 1],
                        vt[64 * j : 64 * j + 64, p * PW + m0 : p * PW + m0 + 128],
                        wlt[64 * j : 64 * j + 64, :],
                    )

            # init: c0 = 0
            for p in range(PAIRS):
                nc.gpsimd.memset(ucs[p][:, PW : 2 * PW], 0.0)
            nc.sync.dma_start(bufs[0][:, 0 : 2 * 2 * PW], xt.ap()[:, 0 : 2 * 2 * PW])
            nc.sync.dma_start(
                bufs[0][:, 2 * 2 * PW :], xt.ap()[:, 2 * 2 * PW : T * 2 * PW]
            )

            vprev = None          # V(t-1) tile
            lpend = None          # (sl, vtile) pending logit emission
            for kc in range(NCH):
                buf = bufs[kc % 2]
                nbuf = bufs[(kc + 1) % 2]
                if kc + 1 < NCH:
                    nxt0 = CS[kc + 1] * 2 * PW
                    nc.sync.dma_start(
                        nbuf[:, 0 : CLEN[kc + 1] * 2 * PW],
                        xt.ap()[:, nxt0 : nxt0 + CLEN[kc + 1] * 2 * PW],
                    )
                for s in range(CLEN[kc]):
                    sl = CS[kc] + s          # local step
                    col0 = s * 2 * PW

                    # --- x-side matmuls for step sl (first step of the
                    # program only; afterwards they are pre-issued below) ---
                    if sl == 0:
                        for p in range(PAIRS):
                            for q in range(4):
                                # start=True clears has_written for the WHOLE
                                # bank; two gates share a bank, so only the
                                # first gate per bank may clear
                                nc.tensor.matmul(
                                    gps[p][:, q * PW : (q + 1) * PW],
                                    wxt[:, q * 128 : (q + 1) * 128],
                                    buf[:, col0 + p * PW : col0 + (p + 1) * PW],
                                    start=(q % 2 == 0), stop=True,
                                )
                    else:
                        # h-side matmuls accumulate onto pre-issued x-side
                        for p in range(PAIRS):
                            for q in range(4):
                                nc.tensor.matmul(
                                    gps[p][:, q * PW : (q + 1) * PW],
                                    wht[:, q * 128 : (q + 1) * 128],
                                    vprev[:, p * PW : (p + 1) * PW],
                                    start=False, stop=True,
                                )

                    # deferred logit matmuls for the previous step (fill the
                    # PE slack while the sigmoid runs)
                    if lpend is not None:
                        emit_logits(*lpend)
                        lpend = None

                    ss = [wpool.tile([128, 4 * PW], BF16, tag=f"s{p}", name=f"s{p}")
                          for p in range(PAIRS)]
                    ms = [wpool.tile([128, 2 * PW], BF16, tag=f"m{p}", name=f"m{p}")
                          for p in range(PAIRS)]
                    tcs = [wpool.tile([128, PW], BF16, tag=f"tc{p}", name=f"tc{p}")
                           for p in range(PAIRS)]
                    vt = vpool.tile([128, 2 * PW], BF16, tag="v", name="v")

                    for p in range(PAIRS):
                        nc.scalar.activation(ss[p][:], gps[p][:], AF.Sigmoid)

                    # --- x-side matmuls for step sl+1 (PSUM freed by sig) ---
                    if sl + 1 < SPC:
                        if s + 1 < CLEN[kc]:
                            xb, xc = buf, (s + 1) * 2 * PW
                        else:
                            xb, xc = nbuf, 0
                        for p in range(PAIRS):
                            for q in range(4):
                                nc.tensor.matmul(
                                    gps[p][:, q * PW : (q + 1) * PW],
                                    wxt[:, q * 128 : (q + 1) * 128],
                                    xb[:, xc + p * PW : xc + (p + 1) * PW],
                                    start=(q % 2 == 0), stop=False,
                                )

                    for p in range(PAIRS):
                        uc = ucs[p]
                        sp = ss[p]
                        # tg = 2*sig(2g) - 1
                        nc.vector.tensor_scalar(
                            uc[:, 0:PW], sp[:, 2 * PW : 3 * PW], 2.0, -1.0,
                            ALU.mult, ALU.add,
                        )
                        # [i*tg | f*c]
                        nc.vector.tensor_tensor(
                            ms[p][:], sp[:, 0 : 2 * PW], uc[:], ALU.mult
                        )
                        # c' = i*tg + f*c
                        nc.vector.tensor_tensor(
                            uc[:, PW : 2 * PW], ms[p][:, 0:PW], ms[p][:, PW : 2 * PW],
                            ALU.add,
                        )
                    for p in range(PAIRS):
                        nc.scalar.activation(
                            tcs[p][:], ucs[p][:, PW : 2 * PW], AF.Tanh
                        )
                    for p in range(PAIRS):
                        nc.vector.tensor_tensor(
                            vt[:, p * PW : (p + 1) * PW],
                            ss[p][:, 3 * PW : 4 * PW], tcs[p][:], ALU.mult,
                        )

                    if sl >= WARM:
                        lpend = (sl, vt)
                    vprev = vt

                    for i, (_, _, trig) in enumerate(PIECES):
                        if trig == sl:
                            emit_gather(i)

            # ---- final step's logits, last gather piece, softmax ----
            # copy/DMA the final piece's ready columns before the last
            # step's logits land, so only a sliver remains on the handoff
            fa, fb, _ = PIECES[2]
            fw = fb - fa
            lsb2 = wpool.tile([128, fw * NG], F32, tag="lsb2", name="lsb2",
                              bufs=1)
            l3 = logits.rearrange("p (g t) -> p g t", g=NG)
            s3 = lsb2.rearrange("p (g t) -> p g t", g=NG)
            c3 = cins[2].rearrange("p (g t) -> p g t", g=NG)
            nc.vector.tensor_copy(s3[:, :, 0 : fw - 1], l3[:, :, fa : fb - 1])
            nc.sync.dma_start(c3[:, :, 0 : fw - 1], s3[:, :, 0 : fw - 1])
            if lpend is not None:
                emit_logits(*lpend)
            nc.vector.tensor_copy(s3[:, :, fw - 1 : fw], l3[:, :, fb - 1 : fb])
            nc.sync.dma_start(c3[:, :, fw - 1 : fw], s3[:, :, fw - 1 : fw])
            # preload the Exp ACT table under the final AllGather; the fake
            # dependency on the last tanh output keeps this AFTER the scan
            # (a dep-free dummy gets hoisted into the head by the scheduler)
            nc.scalar.activation(dum1[:], tcs[1][:, 0:1], AF.Exp)
            nc.gpsimd.collective_compute(
                "AllGather",
                ALU.bypass,
                replica_groups=[[q for q in range(SQ)]],
                ins=[cins[2].opt()],
                outs=[couts[2].opt()],
            )
            for g in range(NG):
                fl3 = fls[g].rearrange("p (q t) -> p q t", q=SQ)
                srci = couts[2].rearrange("(q p) n -> p q n", p=128)[
                    :, :, g * fw : (g + 1) * fw
                ]
                nc.sync.dma_start(fl3[:, :, fa:fb], srci)

            for g in range(NG):
                fl = fls[g]
                ex = wpool.tile([128, N], F32, tag="ex")
                sm = wpool.tile([128, 1], F32, tag="sm")
                rs = wpool.tile([128, 1], F32, tag="rs")
                out = wpool.tile([128, N], F32, tag="out")
                nc.scalar.activation(ex[:], fl[:], AF.Exp, accum_out=sm[:])
                nc.vector.reciprocal(rs[:], sm[:])
                nc.vector.tensor_scalar(out[:], ex[:], rs[:], None, ALU.mult)
                nc.sync.dma_start(y.ap()[g * 128 : (g + 1) * 128, :], out[:])

    nc.compile()
    return nc


def _get_nc():
    if "nc" not in _CACHE:
        _CACHE["nc"] = _build()
    return _CACHE["nc"]


def _prep_weights(W_fc, b_fc, W_ih, W_hh, b_ih, b_hh, W_last):
    Wc = (W_ih @ W_fc).astype(np.float32)                # (256, 30)
    bx = (W_ih @ b_fc + b_ih + b_hh).astype(np.float32)  # (256,)
    Whh = W_hh.astype(np.float32).copy()
    Wc = Wc.copy()
    bx = bx.copy()
    wd = np.full(4 * H, -30.0, dtype=np.float32)         # delta (state reset)
    # PyTorch gate order i,f,g,o; scale g-gate rows by 2 for the sigmoid trick
    Whh[2 * H : 3 * H] *= 2.0
    Wc[2 * H : 3 * H] *= 2.0
    bx[2 * H : 3 * H] *= 2.0
    wd[2 * H : 3 * H] *= 2.0

    whm = np.zeros((128, 4 * 128), dtype=np.float32)
    wxm = np.zeros((2 * XR, 4 * 128), dtype=np.float32)
    for q in range(4):
        rows = slice(q * H, (q + 1) * H)
        wt = Whh[rows].T                                  # (64, 64)
        whm[0:64, q * 128 : q * 128 + 64] = wt
        whm[64:128, q * 128 + 64 : q * 128 + 128] = wt
        xq = np.concatenate(
            [Wc[rows].T, bx[rows][None, :], wd[rows][None, :]], axis=0
        )                                                 # (32, 64)
        wxm[0:XR, q * 128 : q * 128 + 64] = xq
        wxm[XR : 2 * XR, q * 128 + 64 : q * 128 + 128] = xq

    wlb = np.concatenate([W_last.astype(np.float32).T] * 2, axis=0)  # (128, 1)
    return (whm.astype(ml_dtypes.bfloat16), wxm.astype(ml_dtypes.bfloat16),
            np.ascontiguousarray(wlb).astype(ml_dtypes.bfloat16))


def kernel(x, W_fc, b_fc, W_ih, W_hh, b_ih, b_hh, W_last, b_last, _trace=False):
    x = np.asarray(x, dtype=np.float32)
    args = [np.asarray(a, dtype=np.float32) for a in
            (W_fc, b_fc, W_ih, W_hh, b_ih, b_hh, W_last)]
    whm, wxm, wlb = _prep_weights(*args)

    nc = _get_nc()
    in_maps = []
    for c in range(NCORES):
        t0 = OWN * c - WARM
        lo = max(0, -t0)                  # first local step with real data
        xfull = np.zeros((SPC, B, XR), dtype=np.float32)
        xfull[lo:, :, 0:DIN] = x[:, t0 + lo : t0 + SPC].transpose(1, 0, 2)
        xfull[:, :, DIN] = 1.0            # ones row
        xfull[:lo, :, DIN + 1] = 1.0      # delta row: reset state in prefix
        # col (t, p, m); partitions j*32+r
        arr = xfull.reshape(SPC, 2, 2, PW, XR)    # t, p, j, m, row
        arr = arr.transpose(2, 4, 0, 1, 3)        # j, row, t, p, m
        in_maps.append({
            "xt": np.ascontiguousarray(arr).reshape(2 * XR, SPC * 2 * PW)
                    .astype(ml_dtypes.bfloat16),
            "wh": whm, "wx": wxm, "wl": wlb,
        })

    res = run_bass_kernel_spmd(nc, in_maps, list(range(NCORES)), trace=_trace)
    if _trace:
        _CACHE["last_result"] = res
    return res.results[0]["yh"]
